# revision 16
# baseline (speedup 1.0000x reference)
"""Trainium2 Bass kernel for nn_MHA_43095701848407.

MHA forward: qkv = x @ W_qkv, RoPE on q/k, causal softmax attention,
y @ W_proj.  B=4, T=2048, C=2048, 16 heads, head_dim=128, fp32 in/out.

Sharding (8 cores): tensor-parallel over heads (4 shards x 4 heads) x
data-parallel over batch (2 groups x 2 batches).  core = group*4 + shard.

v2: fused streaming schedule, bf16 compute.  Per batch, token slabs of
512 stream through qkv -> RoPE -> causal attention -> partial output
projection with q/k/v kept resident in SBUF (no DRAM bounce).  Slab j's
attention overlaps slab j+1's qkv GEMMs on the PE queue; the projection
for slab j-1 is emitted between them so its normalization broadcasts
have time to land.  Host sums the 4 head-shard partials per batch and
transposes back.

Self-contained: shapes/sharding hardcoded; inputs full-size numpy arrays.
"""

import math
import os
import sys
import types

import numpy as np
import ml_dtypes

import concourse.bass as bass
import concourse.mybir as mybir
import concourse.tile as tile
from concourse import bacc
from concourse.bass_utils import run_bass_kernel_spmd

F32 = mybir.dt.float32
BF16 = mybir.dt.bfloat16
AF = mybir.ActivationFunctionType
ALU = mybir.AluOpType

# Problem shape (hardcoded per contract)
B, T, C = 4, 2048, 2048
H, HD = 16, 128
NCORES = 8
BGROUPS, HSHARDS = 2, 4  # batch groups x head shards
B_LOC = B // BGROUPS  # 2 batches per core
H_LOC = H // HSHARDS  # 4 heads per core
FQK = H_LOC * HD  # 512 features for q (and for k)
FV = H_LOC * HD  # 512 features for v
F_ALL = 3 * H_LOC * HD  # 1536 qkv features per core
KO = C // 128  # 16 contraction chunks
TSLAB = 512
NSLAB = T // TSLAB  # 4 t-slabs per batch
SCALE = 1.0 / math.sqrt(HD)

_CACHED = {}


def _install_ntff_hook():
    """Register the axon NTFF profile hook (container's antenv lacks it)."""
    if "antenv.axon_hooks" in sys.modules:
        return
    try:
        mod = types.ModuleType("antenv.axon_hooks")
        holder = [None]
        mod.set_axon_ntff_profile_hook = lambda h: holder.__setitem__(0, h)
        mod.get_axon_ntff_profile_hook = lambda: holder[0]
        sys.modules["antenv.axon_hooks"] = mod
        import antenv

        antenv.axon_hooks = mod
        if "/root/.axon_site" not in sys.path:
            sys.path.insert(0, "/root/.axon_site")
        from trn_agent_boot.trn_boot import _ntff_profile_via_ctypes

        mod.set_axon_ntff_profile_hook(
            _ntff_profile_via_ctypes("/opt/axon/libaxon_pjrt.so")
        )
    except Exception:
        sys.modules.pop("antenv.axon_hooks", None)


def rope_perm_matrix():
    """lhsT for the rotate-half matmul: rot^T = PT.T @ q^T.
    rot[2i] = -q[2i+1], rot[2i+1] = q[2i]."""
    pt = np.zeros((HD, HD), dtype=np.float32)
    for i in range(HD // 2):
        pt[2 * i + 1, 2 * i] = -1.0
        pt[2 * i, 2 * i + 1] = 1.0
    return pt


def build_nc():
    nc = bacc.Bacc("TRN2", target_bir_lowering=False, debug=False)

    # x pre-sliced host-side: [b, slab, partition, ko, t] so a slab load is
    # one DMA with 16KB contiguous per partition.
    x_sl = nc.dram_tensor(
        "x_sl", [B_LOC, NSLAB, 128, KO, TSLAB], BF16, kind="ExternalInput"
    ).ap()
    w_qkv = nc.dram_tensor("w_qkv", [128, KO, F_ALL], BF16, kind="ExternalInput").ap()
    w_proj = nc.dram_tensor("w_proj", [128, H_LOC, C], BF16, kind="ExternalInput").ap()
    sin_t = nc.dram_tensor("sin_t", [HD, T], BF16, kind="ExternalInput").ap()
    cos_t = nc.dram_tensor("cos_t", [HD, T], BF16, kind="ExternalInput").ap()
    ones_col = nc.dram_tensor("ones_col", [128, 1], BF16, kind="ExternalInput").ap()
    ones_row = nc.dram_tensor("ones_row", [1, 128], BF16, kind="ExternalInput").ap()
    out_t = nc.dram_tensor(
        "out_t", [B_LOC, C // 128, 128, T], F32, kind="ExternalOutput"
    ).ap()

    with tile.TileContext(nc) as tc:
        with nc.allow_low_precision(reason="bf16 compute by design"):
            _emit(nc, tc, x_sl, w_qkv, w_proj, sin_t, cos_t, ones_col, ones_row, out_t)
    nc.compile()
    return nc


def _emit(nc, tc, x_sl, w_qkv, w_proj, sin_t, cos_t, ones_col, ones_row, out_t):
    with (
        tc.tile_pool(name="consts", bufs=1) as consts,
        tc.tile_pool(name="wpool", bufs=1) as wpool,
        tc.tile_pool(name="xpool", bufs=2) as xpool,
        tc.tile_pool(name="qpool", bufs=2) as qpool,
        tc.tile_pool(name="kpool", bufs=NSLAB + 1) as kpool,
        tc.tile_pool(name="vpool", bufs=NSLAB + 1) as vpool,
        tc.tile_pool(name="ropepool", bufs=8) as ropepool,
        tc.tile_pool(name="ppool", bufs=8) as ppool,
        tc.tile_pool(name="ypool", bufs=6) as ypool,
        tc.tile_pool(name="npool", bufs=8) as npool,
        tc.tile_pool(name="lpool", bufs=2) as lpool,
        tc.tile_pool(name="opool", bufs=2) as opool,
        tc.tile_pool(name="mmps", bufs=2, space="PSUM") as mmps,
        tc.tile_pool(name="sps", bufs=2, space="PSUM") as sps,
        tc.tile_pool(name="yps", bufs=2, space="PSUM") as yps,
        tc.tile_pool(name="lops", bufs=2, space="PSUM") as lops,
        tc.tile_pool(name="nbounce", bufs=4, space="DRAM") as nbounce,
    ):
        steps = [(b, j) for b in range(B_LOC) for j in range(NSLAB)]

        # x slab loads on the scalar queue (w/consts go on sync so the two
        # startup streams run in parallel); first slab ko-split for fast start
        x_tiles = {}

        def load_x(b, j, split):
            t_ = xpool.tile([128, KO, TSLAB], BF16, name="x_sb")
            if split:
                for ko in range(KO):
                    nc.sync.dma_start(t_[:, ko, :], x_sl[b, j, :, ko, :])
            else:
                nc.sync.dma_start(t_, x_sl[b, j])
            x_tiles[(b, j)] = t_

        load_x(0, 0, True)

        # small consts first (rope needs sin/cos before the first slab ends),
        # then weights ko-split so the first matmuls start early
        sin_sb = consts.tile([HD, T], BF16)
        nc.scalar.dma_start(sin_sb, sin_t)
        cos_sb = consts.tile([HD, T], BF16)
        nc.scalar.dma_start(cos_sb, cos_t)
        ones_sb = consts.tile([128, 1], BF16)
        nc.scalar.dma_start(ones_sb, ones_col)
        ones_row_sb = consts.tile([1, 128], BF16)
        nc.scalar.dma_start(ones_row_sb, ones_row)
        w_sb = wpool.tile([128, KO, F_ALL], BF16)
        for ko in range(KO):
            nc.scalar.dma_start(w_sb[:, ko, :], w_qkv[:, ko, :])
        wp_sb = wpool.tile([128, H_LOC, C], BF16)
        nc.scalar.dma_start(wp_sb, w_proj)

        load_x(0, 1, False)

        k_tiles = {}
        v_tiles = {}
        # per (b, j): list of (y_sb, bc_sb) for deferred normalization
        pending_proj = {}

        def emit_qkv(b, j):
            x_sb = x_tiles.pop((b, j))
            tsl = slice(j * TSLAB, (j + 1) * TSLAB)
            q_sb = qpool.tile([128, H_LOC, TSLAB], BF16, name="q_sb")
            k_sb = kpool.tile([128, H_LOC, TSLAB], BF16, name="k_sb")
            k_tiles[(b, j)] = k_sb

            def emit_rope(f, raw):
                # rotate-half via two partition-pair-swap SBUF->SBUF DMAs; the
                # sign lives in host-negated sinA rows (no PE work at all)
                rawsw = ropepool.tile([128, TSLAB], BF16, name="rawsw", tag="rp")
                r3 = raw.rearrange("(h two) f -> h two f", two=2)
                w3 = rawsw.rearrange("(h two) f -> h two f", two=2)
                nc.sync.dma_start(w3[:, 0, :], r3[:, 1, :])
                nc.sync.dma_start(w3[:, 1, :], r3[:, 0, :])
                # roped = raw*cos + swapped*sinA
                t1 = ropepool.tile([128, TSLAB], BF16, name="t1", tag="rp")
                nc.vector.tensor_tensor(t1, raw, cos_sb[:, tsl], ALU.mult)
                t2 = ropepool.tile([128, TSLAB], BF16, name="t2", tag="rp")
                nc.vector.tensor_tensor(t2, rawsw, sin_sb[:, tsl], ALU.mult)
                if f < H_LOC:
                    dest = q_sb[:, f, :]
                else:
                    dest = k_sb[:, f - H_LOC, :]
                nc.vector.tensor_tensor(dest, t1, t2, ALU.add)

            for f in range(2 * H_LOC):
                ps = mmps.tile([128, TSLAB], F32, name="qk_ps", tag="mm")
                for ko in range(KO):
                    nc.tensor.matmul(
                        ps,
                        w_sb[:, ko, f * 128 : (f + 1) * 128],
                        x_sb[:, ko, :],
                        start=(ko == 0),
                        stop=(ko == KO - 1),
                    )
                raw = ropepool.tile([128, TSLAB], BF16, name="raw", tag="rp")
                nc.vector.tensor_copy(raw, ps)
                emit_rope(f, raw)
            v_sb = vpool.tile([128, TSLAB // 128, FV], BF16, name="v_sb")
            v_tiles[(b, j)] = v_sb
            for tb in range(TSLAB // 128):
                ps = mmps.tile([128, FV], F32, name="v_ps", tag="mm")
                for ko in range(KO):
                    nc.tensor.matmul(
                        ps,
                        x_sb[:, ko, tb * 128 : (tb + 1) * 128],
                        w_sb[:, ko, 2 * FQK : 2 * FQK + FV],
                        start=(ko == 0),
                        stop=(ko == KO - 1),
                    )
                nc.vector.tensor_copy(v_sb[:, tb, :], ps)
            return q_sb

        def emit_attn(b, j, q_sb):
            """Two heads in lockstep, l/y one block behind s/exp: exp and the
            causal select always have a full block of s-matmuls to hide in."""
            nkb = 4 * (j + 1)
            pairs = [None] * H_LOC
            for h0 in range(0, H_LOC, 2):
                hs = (h0, h0 + 1)
                y_ps = {h: yps.tile([HD, TSLAB], F32, name="y_ps") for h in hs}
                l_ps = {h: lops.tile([1, TSLAB], F32, name="l_ps", bufs=2) for h in hs}
                prev = None

                def emit_ly(kb, p_of):
                    kslab, kin = kb // 4, kb % 4
                    qoff = 128 * kin if kslab == j else 0
                    v_sb = v_tiles[(b, kslab)]
                    for h in hs:
                        nc.tensor.matmul(
                            l_ps[h][:, qoff:],
                            ones_sb,
                            p_of[h][:, qoff:],
                            start=(kb == 0),
                            stop=(kb == nkb - 1),
                        )
                        nc.tensor.matmul(
                            y_ps[h][:, qoff:],
                            v_sb[:, kin, h * 128 : (h + 1) * 128],
                            p_of[h][:, qoff:],
                            start=(kb == 0),
                            stop=(kb == nkb - 1),
                        )

                for kb in range(nkb):
                    kslab, kin = kb // 4, kb % 4
                    diag = kslab == j
                    qoff = 128 * kin if diag else 0
                    qn = TSLAB - qoff
                    k_sb = k_tiles[(b, kslab)]
                    p_of = {}
                    for h in hs:
                        s_ps = sps.tile([128, TSLAB], F32, name="s_ps", tag="s")
                        nc.tensor.matmul(
                            s_ps[:, qoff:],
                            k_sb[:, h, kin * 128 : (kin + 1) * 128],
                            q_sb[:, h, qoff:],
                            start=True,
                            stop=True,
                        )
                        p_sb = ppool.tile([128, TSLAB], BF16, name="p_sb")
                        nc.scalar.activation(
                            p_sb[:, qoff:], s_ps[:, qoff:], AF.Exp, scale=SCALE
                        )
                        if diag:
                            # causal: keep where (q - qoff) - k >= 0
                            nc.gpsimd.affine_select(
                                out=p_sb[:, qoff:],
                                in_=p_sb[:, qoff:],
                                pattern=[[1, qn]],
                                compare_op=ALU.is_ge,
                                fill=0.0,
                                base=0,
                                channel_multiplier=-1,
                            )
                        p_of[h] = p_sb
                    if prev is not None:
                        emit_ly(*prev)
                    prev = (kb, p_of)
                emit_ly(*prev)

                last_step = (b, j) == (B_LOC - 1, NSLAB - 1)
                for h in hs:
                    y_sb = ypool.tile([HD, TSLAB], BF16, name="y_sb")
                    nc.vector.tensor_copy(y_sb, y_ps[h])
                    linv = lpool.tile([1, TSLAB], F32, name="linv")
                    nc.vector.reciprocal_approx_fast(linv, l_ps[h])
                    if last_step:
                        # final slab: broadcast 1/l via a PE matmul (the DRAM
                        # bounce's round trip would stall the last projection)
                        linv_bf = lpool.tile([1, TSLAB], BF16, name="linv_bf", tag="lbf", bufs=4)
                        nc.vector.tensor_copy(linv_bf, linv)
                        pairs[h] = (y_sb, linv_bf)
                    else:
                        linv_dr = nbounce.tile([1, TSLAB], F32, name="linv_dr")
                        nc.sync.dma_start(linv_dr, linv)
                        bc_sb = npool.tile([128, TSLAB], F32, name="bc_sb")
                        nc.sync.dma_start(bc_sb, linv_dr.to_broadcast([128, TSLAB]))
                        pairs[h] = (y_sb, bc_sb)
            pending_proj[(b, j)] = pairs

        def emit_proj(b, j):
            pairs = pending_proj.pop((b, j))
            last_step = (b, j) == (B_LOC - 1, NSLAB - 1)
            # normalize: y /= l (broadcasts launched during attention)
            for y_sb, bc_sb in pairs:
                if last_step:
                    bc_ps = sps.tile([128, TSLAB], F32, name="bc_ps", tag="s")
                    nc.tensor.matmul(bc_ps, ones_row_sb, bc_sb, start=True, stop=True)
                    nc.vector.tensor_tensor(y_sb, y_sb, bc_ps, ALU.mult)
                else:
                    nc.gpsimd.tensor_tensor(y_sb, y_sb, bc_sb, ALU.mult)
            tsl = slice(j * TSLAB, (j + 1) * TSLAB)
            for co in range(C // 128):
                o_ps = mmps.tile([128, TSLAB], F32, name="o_ps", tag="mm")
                for h in range(H_LOC):
                    nc.tensor.matmul(
                        o_ps,
                        wp_sb[:, h, co * 128 : (co + 1) * 128],
                        pairs[h][0],
                        start=(h == 0),
                        stop=(h == H_LOC - 1),
                    )
                o_sb = opool.tile([128, TSLAB], F32, name="o_sb")
                if co % 2 == 0:
                    nc.vector.tensor_copy(o_sb, o_ps)
                else:
                    nc.scalar.copy(o_sb, o_ps)
                nc.sync.dma_start(out_t[b, co, :, tsl], o_sb)

        for si, (b, j) in enumerate(steps):
            if si + 2 < len(steps):
                load_x(*steps[si + 2], False)
            q_sb = emit_qkv(b, j)
            if si > 0:
                emit_proj(*steps[si - 1])
            emit_attn(b, j, q_sb)
        emit_proj(*steps[-1])


def _get_nc():
    if "nc" not in _CACHED:
        _CACHED["nc"] = build_nc()
    return _CACHED["nc"]


def kernel(x, sin, cos, W_qkv, W_proj):
    x = np.asarray(x, dtype=np.float32)
    sin = np.asarray(sin, dtype=np.float32)
    cos = np.asarray(cos, dtype=np.float32)
    W_qkv = np.asarray(W_qkv, dtype=np.float32)
    W_proj = np.asarray(W_proj, dtype=np.float32)
    bf = ml_dtypes.bfloat16

    sin_a = np.ascontiguousarray(sin[0, 0].T)  # [HD, T]
    sin_a = sin_a.copy()
    sin_a[0::2, :] *= -1.0
    sin_t = sin_a.astype(bf)
    cos_t = np.ascontiguousarray(cos[0, 0].T).astype(bf)
    ones_col = np.ones((128, 1), bf)
    ones_row = np.ones((1, 128), bf)

    in_maps = []
    for g in range(BGROUPS):
        # x^T [b, C, T] -> [b, slab, p, ko, t] with C = ko*128 + p
        x_g = x[g * B_LOC : (g + 1) * B_LOC]  # [B_LOC, T, C]
        x_t = x_g.transpose(0, 2, 1).reshape(B_LOC, KO, 128, NSLAB, TSLAB)
        x_pre = np.ascontiguousarray(x_t.transpose(0, 3, 2, 1, 4)).astype(bf)
        for s in range(HSHARDS):
            qcols = W_qkv[:, s * FQK : (s + 1) * FQK]
            kcols = W_qkv[:, C + s * FQK : C + (s + 1) * FQK]
            vcols = W_qkv[:, 2 * C + s * FV : 2 * C + (s + 1) * FV]
            w_loc = np.concatenate([qcols, kcols, vcols], axis=1)  # [C, F_ALL]
            w_pre = np.ascontiguousarray(
                w_loc.reshape(KO, 128, F_ALL).transpose(1, 0, 2)
            ).astype(bf)
            wp_loc = W_proj[s * FV : (s + 1) * FV, :]  # [FV, C]
            wp_pre = np.ascontiguousarray(
                wp_loc.reshape(H_LOC, 128, C).transpose(1, 0, 2)
            ).astype(bf)
            in_maps.append(
                {
                    "x_sl": x_pre,
                    "w_qkv": w_pre,
                    "w_proj": wp_pre,
                    "sin_t": sin_t,
                    "cos_t": cos_t,
                    "ones_col": ones_col,
                    "ones_row": ones_row,
                }
            )

    trace = bool(int(os.environ.get("KERNEL_TRACE", "0")))
    if trace:
        _install_ntff_hook()
    nc = _get_nc()
    res = run_bass_kernel_spmd(
        nc, in_maps, core_ids=list(range(NCORES)), trace=trace
    )
    _CACHED["last_result"] = res

    out = np.zeros((B, T, C), dtype=np.float32)
    for g in range(BGROUPS):
        acc = np.zeros((B_LOC, C // 128, 128, T), dtype=np.float32)
        for s in range(HSHARDS):
            acc += res.results[g * HSHARDS + s]["out_t"]
        # [b, co, p, t] -> [b, t, co*128+p]
        out[g * B_LOC : (g + 1) * B_LOC] = acc.reshape(B_LOC, C, T).transpose(
            0, 2, 1
        )
    return out
